# revision 8
# baseline (speedup 1.0000x reference)
"""Trainium2 Bass kernel for nn_MultiHeadODELinear.

Math: out = sum_{k=0..4} (t^k/k!) blockdiag(A_h)^k (x @ W.T + b)
The Taylor loop commutes with the token dimension, so it folds into the
projection:  out = x @ W_eff.T + b_eff  with
  W_eff = E @ W,  b_eff = E @ b,  E = blockdiag(M_h),
  M_h  = sum_{k=0..4} (t^k/k!) A_h^k   (16 heads of 64x64).

Per-core work (data-parallel over batch, 1 batch of [4096, 1024] per core):
  phase 0 (small): build N = blockdiag(M_h^T) via a Horner recurrence of
    PE matmuls, then WT_eff[d, o] = sum_m W[m, d] * N[m, o] with W chunks as
    lhsT (natural layout), plus b_eff broadcast tile.
  phase 1 (main): per 128-token tile: DMA x, PE-transpose to get d on
    partitions, then 16 accumulating f32r matmuls against WT_eff, add bias
    on copyback, DMA out.
"""

import sys

for _p in ("/opt/trn_rl_repo",):
    if _p not in sys.path:
        sys.path.insert(0, _p)

import numpy as np

import concourse.bass as bass  # noqa: F401
import concourse.tile as tile
from concourse import bacc, mybir
from concourse import bass_utils
from concourse.masks import make_identity

F32 = mybir.dt.float32
F32R = mybir.dt.float32r

B, S, D = 8, 4096, 1024
H, HD = 16, 64
ORDERS = 4
P = 128
NCHUNK = D // P          # 8 chunks of 128 along any 1024 dim
TTILES = S // P          # 32 token tiles per core
N_CORES = 8

_NC_CACHE = {}


def _build_nc(repeats=1):
    nc = bacc.Bacc("TRN2", target_bir_lowering=False, debug=False)

    x_d = nc.dram_tensor("x", [S, D], F32, kind="ExternalInput").ap()
    w_d = nc.dram_tensor("W", [D, D], F32, kind="ExternalInput").ap()
    b_d = nc.dram_tensor("b", [D], F32, kind="ExternalInput").ap()
    a_d = nc.dram_tensor("A", [H, HD, HD], F32, kind="ExternalInput").ap()
    t_d = nc.dram_tensor("t", [1, 1], F32, kind="ExternalInput").ap()
    o_d = nc.dram_tensor("out", [S, D], F32, kind="ExternalOutput").ap()

    with tile.TileContext(nc) as tc:
        with tc.tile_pool(name="const", bufs=1) as const_pool, \
             tc.tile_pool(name="wsb", bufs=1) as w_pool, \
             tc.tile_pool(name="xin", bufs=3) as x_pool, \
             tc.tile_pool(name="xt", bufs=2) as xt_pool, \
             tc.tile_pool(name="osb", bufs=3) as o_pool, \
             tc.tile_pool(name="ps_small", bufs=2, space="PSUM") as ps_small, \
             tc.tile_pool(name="ps_t", bufs=2, space="PSUM") as ps_t, \
             tc.tile_pool(name="ps_o", bufs=2, space="PSUM") as ps_o:

            # ---------------- phase 0: W_eff / b_eff ----------------
            ident = const_pool.tile([P, P], F32)
            make_identity(nc, ident[:])

            # t coefficient vectors c_k = t^k/k! as [128, 1] per-partition
            t_sb = const_pool.tile([1, 1], F32)
            nc.sync.dma_start(t_sb[:], t_d[:])
            ones_row = const_pool.tile([1, P], F32)
            nc.vector.memset(ones_row[:], 1.0)
            ones_r = const_pool.tile([1, P], F32R)
            nc.vector.tensor_copy(ones_r[:], ones_row[:])

            ps_tv = ps_small.tile([P, 512], F32, tag="ps0", name="ps_tv")
            nc.tensor.matmul(ps_tv[:, 0:1], ones_row[:], t_sb[:], start=True, stop=True)
            c1 = const_pool.tile([P, 1], F32)
            nc.vector.tensor_copy(c1[:], ps_tv[:, 0:1])
            c2 = const_pool.tile([P, 1], F32)
            nc.vector.tensor_tensor(c2[:], c1[:], c1[:], mybir.AluOpType.mult)
            nc.vector.tensor_scalar_mul(c2[:], c2[:], 0.5)
            c3 = const_pool.tile([P, 1], F32)
            nc.vector.tensor_tensor(c3[:], c2[:], c1[:], mybir.AluOpType.mult)
            nc.vector.tensor_scalar_mul(c3[:], c3[:], 1.0 / 3.0)
            c4 = const_pool.tile([P, 1], F32)
            nc.vector.tensor_tensor(c4[:], c3[:], c1[:], mybir.AluOpType.mult)
            nc.vector.tensor_scalar_mul(c4[:], c4[:], 0.25)

            # scaled identities c_k * I
            cI = []
            for ck in (c1, c2, c3, c4):
                ckI = const_pool.tile([P, P], F32, tag=f"cI{len(cI)}")
                nc.vector.tensor_scalar(ckI[:], ident[:], ck[:], None,
                                        mybir.AluOpType.mult)
                cI.append(ckI)
            c1I, c2I, c3I, c4I = cI

            # A as per-chunk block-diagonal pairs: A_blk[:, c, :] holds
            # A[2c] in [0:64, 0:64] and A[2c+1] in [64:128, 64:128].
            a_blk = const_pool.tile([P, NCHUNK, P], F32)
            nc.gpsimd.memset(a_blk[:], 0.0)
            for h in range(H):
                r0 = (h % 2) * HD
                nc.sync.dma_start(
                    a_blk[r0:r0 + HD, h // 2, r0:r0 + HD], a_d[h])

            # Horner: S <- A_c^T S + c_k I, starting from rhs = c4*I.
            # After 4 steps S = blockdiag(M_h^T) restricted to chunk c.
            # Final step writes into N2 (f32r, [128, c, 256] with the block at
            # column half c%2 so 256-wide o-bands pair two chunks).
            n2 = const_pool.tile([P, NCHUNK, 2 * P], F32R)
            nc.gpsimd.memset(n2[:].bitcast(F32), 0.0)
            s_prev = None
            for step in range(ORDERS):
                if step < ORDERS - 1:
                    s_new = const_pool.tile([P, NCHUNK, P], F32, tag=f"S{step}")
                for c in range(NCHUNK):
                    ps_s = ps_small.tile([P, 512], F32, tag="ps0", name="ps_s")
                    ps_s = ps_s[:, 0:P]
                    rhs = c4I[:] if step == 0 else s_prev[:, c, :]
                    nc.tensor.matmul(ps_s[:], a_blk[:, c, :], rhs,
                                     start=True, stop=True)
                    addI = (c3I, c2I, c1I)[step] if step < ORDERS - 1 else ident
                    if step < ORDERS - 1:
                        nc.vector.tensor_tensor(s_new[:, c, :], ps_s[:], addI[:],
                                                mybir.AluOpType.add)
                    else:
                        col = (c % 2) * P
                        nc.vector.tensor_tensor(n2[:, c, col:col + P], ps_s[:],
                                                addI[:], mybir.AluOpType.add)
                if step < ORDERS - 1:
                    s_prev = s_new

            # W in natural layout, rounded to f32r for the WT_eff matmuls
            w_sb = w_pool.tile([P, NCHUNK, D], F32)
            nc.sync.dma_start(w_sb[:], w_d.rearrange("(c p) d -> p c d", p=P))
            w_r = w_pool.tile([P, NCHUNK, D], F32R)
            nc.vector.tensor_copy(w_r[:], w_sb[:])

            # WT_eff[d, o] = sum_m W[m, d] N[m, o]; o in 256-wide bands
            # (chunk pair cp), m runs over chunks 2cp, 2cp+1.
            wte = w_pool.tile([P, NCHUNK, D], F32R)
            for dc in range(NCHUNK):
                for cp in range(NCHUNK // 2):
                    ps_w = ps_small.tile([P, 512], F32, tag="ps0", name="ps_w")
                    ps_w = ps_w[:, 0:2 * P]
                    nc.tensor.matmul(ps_w[:], w_r[:, 2 * cp, dc * P:(dc + 1) * P],
                                     n2[:, 2 * cp, :], start=True, stop=False)
                    nc.tensor.matmul(ps_w[:], w_r[:, 2 * cp + 1, dc * P:(dc + 1) * P],
                                     n2[:, 2 * cp + 1, :], start=False, stop=True)
                    nc.scalar.mul(wte[:, dc, cp * 2 * P:(cp + 1) * 2 * P],
                                  ps_w[:], 1.0)

            # b_eff = N^T b, assembled as a [1, 1024] row then broadcast
            b_f32 = const_pool.tile([P, NCHUNK], F32)
            nc.sync.dma_start(b_f32[:], b_d.rearrange("(c p) -> p c", p=P))
            b_sb = const_pool.tile([P, NCHUNK], F32R)
            nc.vector.tensor_copy(b_sb[:], b_f32[:])
            b_row = const_pool.tile([1, D], F32R)
            for cp in range(NCHUNK // 2):
                ps_b = ps_small.tile([P, 512], F32, tag="ps0", name="ps_b")
                ps_b = ps_b[0:1, 0:2 * P]
                nc.tensor.matmul(ps_b[:], b_sb[:, 2 * cp:2 * cp + 1],
                                 n2[:, 2 * cp, :], start=True, stop=False)
                nc.tensor.matmul(ps_b[:], b_sb[:, 2 * cp + 1:2 * cp + 2],
                                 n2[:, 2 * cp + 1, :], start=False, stop=True)
                nc.vector.tensor_copy(b_row[:, cp * 2 * P:(cp + 1) * 2 * P], ps_b[:])
            b_bcast = const_pool.tile([P, D], F32)
            for half in range(2):
                ps_bb = ps_small.tile([P, 512], F32, tag="ps0", name="ps_bb")
                nc.tensor.matmul(ps_bb[:], ones_r[:],
                                 b_row[:, half * 512:(half + 1) * 512],
                                 start=True, stop=True)
                nc.scalar.mul(b_bcast[:, half * 512:(half + 1) * 512], ps_bb[:], 1.0)

            # ---------------- phase 1: main loop ----------------
            for tt in range(TTILES * repeats):
                tt = tt % TTILES
                x_t = x_pool.tile([P, D], F32)
                nc.sync.dma_start(x_t[:], x_d[tt * P:(tt + 1) * P, :])

                # transpose x tile: d onto partitions (f32r on copyback)
                xt = xt_pool.tile([P, NCHUNK, P], F32R)
                for g in range(2):
                    ps_tr = ps_t.tile([P, 512], F32)
                    for q in range(4):
                        dc = g * 4 + q
                        nc.tensor.transpose(ps_tr[:, q * P:(q + 1) * P],
                                            x_t[:, dc * P:(dc + 1) * P], ident[:])
                    nc.scalar.mul(xt[:, g * 4:(g + 1) * 4, :], ps_tr[:], 1.0)

                # out[t, o] = sum_dc xt[:, dc, :]^T @ wte[:, dc, o]
                ps_out = [ps_o.tile([P, 512], F32, tag=f"ps_out{oh}",
                                    name=f"ps_out{oh}")
                          for oh in range(2)]
                for dc in range(NCHUNK):
                    for oh in range(2):
                        nc.tensor.matmul(ps_out[oh][:], xt[:, dc, :],
                                         wte[:, dc, oh * 512:(oh + 1) * 512],
                                         start=(dc == 0), stop=(dc == NCHUNK - 1))
                o_sb = o_pool.tile([P, D], F32)
                for oh in range(2):
                    nc.vector.tensor_tensor(o_sb[:, oh * 512:(oh + 1) * 512],
                                            ps_out[oh][:],
                                            b_bcast[:, oh * 512:(oh + 1) * 512],
                                            mybir.AluOpType.add)
                nc.sync.dma_start(o_d[tt * P:(tt + 1) * P, :], o_sb[:])

    nc.compile()
    return nc


def get_nc(repeats=1):
    if repeats not in _NC_CACHE:
        _NC_CACHE[repeats] = _build_nc(repeats)
    return _NC_CACHE[repeats]


def make_in_maps(x, t_scalar, W, b, A):
    x = np.ascontiguousarray(np.asarray(x, dtype=np.float32))
    t = np.asarray(t_scalar, dtype=np.float32).reshape(1, 1)
    W = np.ascontiguousarray(np.asarray(W, dtype=np.float32))
    b = np.ascontiguousarray(np.asarray(b, dtype=np.float32))
    A = np.ascontiguousarray(np.asarray(A, dtype=np.float32))
    return [{"x": x[i], "W": W, "b": b, "A": A, "t": t} for i in range(N_CORES)]


def kernel(x, t_scalar, W, b, A):
    nc = get_nc()
    in_maps = make_in_maps(x, t_scalar, W, b, A)
    res = bass_utils.run_bass_kernel_spmd(nc, in_maps, core_ids=list(range(N_CORES)))
    return np.stack([res.results[i]["out"] for i in range(N_CORES)], axis=0)


if __name__ == "__main__":
    rng = np.random.default_rng(0)
    x = rng.standard_normal((B, S, D), dtype=np.float32)
    W = rng.standard_normal((D, D), dtype=np.float32) / 32.0
    b = rng.standard_normal((D,), dtype=np.float32) * 0.01
    A = rng.standard_normal((H, HD, HD), dtype=np.float32) * 0.02
    t = np.float32(0.6)
    out = kernel(x, t, W, b, A)
    print("out", out.shape, out.dtype)
